# revision 15
# baseline (speedup 1.0000x reference)
"""Trainium2 Bass kernel for the pairwise-score attention + gated MLP encoding.

Computation (per batch element b, p=1024 tokens, d=256 features):
    A[i,j]  = wa.P_i + wb.P_j + (P_i*wc).P_j
    itr     = softmax_j(A) @ P
    cat     = [P, itr]
    z       = tanh(cat@w1+b1); r = sigmoid(cat@w2+b2); f = sigmoid(cat@w3+b3)
    out     = r*P + f*z

Sharding: data-parallel over batch across 8 NeuronCores (4 batch el / core).

v3 kernel structure (per batch element):
  - P shipped bf16 (for P^T via XBAR DMA transposes straight from DRAM) and
    fp8e4 (natural layout, the itr-matmul stationary).  The wa.P_i term is
    constant along the softmax axis j and cancels -> never computed.
  - Score + attention matmuls in fp8e4 DoubleRow (K=256 per pass, fp8 peak):
    S^T = PT8.T @ PcT8 per j-chunk, exp on ACT writes fp8 pair tiles, softmax
    denominator via an all-ones fp8 stationary, itr^T numerator with
    stationary natural-fp8 j-chunk pairs.  sb[j] = P_j.wb via 8 tiny DR
    matvecs off PT8, used as the exp bias (with a -2 overflow-safety shift
    that cancels in the softmax).
  - MLP in bf16, transposed (out^T = (cat@w)^T) so b1/b2/b3 are per-partition
    ACT biases; sigmoid as 0.5+0.5*tanh(x/2) keeps one ACT table set.
  - Gating bf16: out = (t2+1)*(P/2)+0.5*[(t3+1)*z]; the middle product runs
    on GpSimd, the rest on DVE.
  - Output transposed back on the PE in bf16, stored bf16 via the GpSimd DMA
    queue; host casts to f32.
  - The emission interleaves scores(b+1)+exp(b+1) into mlp(b) so the ACT
    queue (14 activates/batch) and the PE stay co-saturated, and out(b-1)
    transpose pairs fill the remaining PE bubbles.  Dummy PE transposes at
    the head keep HAM from idling back to the cold clock.
"""

import sys

if "/opt/trn_rl_repo" not in sys.path:
    sys.path.insert(0, "/opt/trn_rl_repo")

import numpy as np
import ml_dtypes

import concourse.bass as bass
import concourse.mybir as mybir
import concourse.tile as tile
from concourse import bacc
from concourse.bass_utils import run_bass_kernel_spmd
from concourse.masks import make_identity

F32 = mybir.dt.float32
BF16 = mybir.dt.bfloat16
FP8 = mybir.dt.float8e4
AF = mybir.ActivationFunctionType
ALU = mybir.AluOpType
DRM = mybir.MatmulPerfMode.DoubleRow

B, PLEN, D = 32, 1024, 256
N_CORES = 8
B_LOC = B // N_CORES  # batch elements per core

NJ = PLEN // 128  # 8 token chunks of 128
ND = D // 128     # 2 feature chunks of 128
NPAIR = NJ // 2   # 4 token-chunk pairs (fp8 DoubleRow K=256)


def _emit(ctx, tc, P_in, P8_in, wb_in, wc_in, w_mlp, b_mlp, out):
    nc = tc.nc
    ts = bass.ts

    const = ctx.enter_context(tc.tile_pool(name="const", bufs=1))
    pin = ctx.enter_context(tc.tile_pool(name="pin", bufs=2))
    ptp = ctx.enter_context(tc.tile_pool(name="ptp", bufs=2))
    pexp = ctx.enter_context(tc.tile_pool(name="pexp", bufs=2))
    pitr = ctx.enter_context(tc.tile_pool(name="pitr", bufs=2))
    pmlp = ctx.enter_context(tc.tile_pool(name="pmlp", bufs=2))
    pout = ctx.enter_context(tc.tile_pool(name="pout", bufs=1))
    # PSUM is 8 banks: ps_big 3x2 + pst 1 + psb 1
    ps_big = ctx.enter_context(tc.tile_pool(name="ps_big", bufs=3, space="PSUM"))
    ps_t2 = ctx.enter_context(tc.tile_pool(name="ps_t2", bufs=1, space="PSUM"))
    ps_sb = ctx.enter_context(tc.tile_pool(name="ps_sb", bufs=1, space="PSUM"))

    as3 = lambda ap: ap.rearrange("p (c x) -> p c x", c=2)

    # ---- constants (once per core) ----
    ident = const.tile([128, 128], F32)
    make_identity(nc, ident)
    ident_bf = const.tile([128, 128], BF16)
    nc.vector.tensor_copy(out=ident_bf, in_=ident)
    ones_f = const.tile([128, 256], F32)
    nc.vector.memset(ones_f, 1.0)
    ones8 = const.tile([128, 256], FP8)  # as3 -> [128, 2, 128] DR stationary
    nc.vector.tensor_copy(out=ones8, in_=ones_f)

    # const weight DMAs are emitted inside the prologue on the sync queue,
    # ordered by first use (see _prologue_consts)
    wb_sb, wc_sb, w_sb, b_sb = [], [], [], []

    def _consts_early():
        # wb/wc: needed by the first sb matvecs / PcT8 cast
        for dc in range(ND):
            t = const.tile([128, 1], BF16, tag=f"wb{dc}")
            nc.sync.dma_start(out=t, in_=wb_in[bass.ds(dc * 128, 128)].unsqueeze(1))
            wb_sb.append(t)
        for dc in range(ND):
            t = const.tile([128, 1], F32, tag=f"wc{dc}")
            nc.sync.dma_start(out=t, in_=wc_in[bass.ds(dc * 128, 128)].unsqueeze(1))
            wc_sb.append(t)

    def _consts_late():
        # MLP weights (bf16): [512, 256] -> sbuf [128, 4(kc), 256]
        for wi in range(3):
            wt = const.tile([128, 4, D], BF16, tag=f"w{wi}")
            nc.sync.dma_start(
                out=wt, in_=w_mlp[wi].rearrange("(kc k) d -> k kc d", k=128))
            w_sb.append(wt)
        # biases, per dout-chunk [128,1]; for r/f (sigmoid-via-tanh): b/2
        for wi in range(3):
            chunks = []
            for dc in range(ND):
                bt = const.tile([128, 1], F32, tag=f"b{wi}{dc}")
                nc.sync.dma_start(out=bt,
                                  in_=b_mlp[wi][bass.ds(dc * 128, 128)].unsqueeze(1))
                if wi > 0:
                    bh = const.tile([128, 1], F32, tag=f"bh{wi}{dc}")
                    nc.scalar.mul(out=bh, in_=bt, mul=0.5)
                    bt = bh
                chunks.append(bt)
            b_sb.append(chunks)

    # ---- helpers ----
    def dummy_mm(k):
        """Keep-warm PE transpose with no data dependencies."""
        t = ps_t2.tile([128, 256], BF16, tag="pst", name=f"dum{k}")
        nc.tensor.transpose(t[:, 0:128], ident_bf, ident_bf)

    def phase_load(b):
        """fp8 natural tile + XBAR bf16 transposes, all on the sync queue."""
        P8 = pin.tile([128, NJ, D], FP8, tag="p8", name="P8")
        nc.sync.dma_start(out=P8,
                          in_=P8_in[b].rearrange("(jc k) d -> k jc d", k=128))
        # bufs=3: PT is written at iter-(b-1) start and read through iter b's
        # mlp, so a 2-deep rotation would stall the next XBAR on mlp(b-1)
        PT = ptp.tile([128, 2 * PLEN], BF16, tag="pt", name="PT", bufs=3)
        for dc in range(ND):
            nc.sync.dma_start_transpose(out=PT[:, ts(dc, PLEN)],
                                        in_=P_in[b, :, ts(dc, 128)])
        return P8, PT

    def prep_casts(PT):
        """PT8 + PcT8 on DVE (early; feeds sb matvecs and scores)."""
        PT8 = ptp.tile([128, 2 * PLEN], FP8, tag="pt8", name="PT8")
        nc.vector.tensor_copy(out=PT8, in_=PT)
        PcT8 = ptp.tile([128, 2 * PLEN], FP8, tag="pct8", name="PcT8")
        for dc in range(ND):
            nc.vector.tensor_scalar_mul(out=PcT8[:, ts(dc, PLEN)],
                                        in0=PT[:, ts(dc, PLEN)],
                                        scalar1=wc_sb[dc])
        return PT8, PcT8

    def prep_ph(PT):
        Ph = ptp.tile([128, 2 * PLEN], BF16, tag="ph", name="Ph")
        nc.vector.tensor_scalar_mul(out=Ph, in0=PT, scalar1=0.5)
        return Ph

    def phase_sb(PT):
        """sb[j] = P_j.wb - 2 via 16 bf16 matvecs; evac as the exp bias tile."""
        psb = ps_sb.tile([128, NJ], F32, tag="psb", name="psb")
        for jc in range(NJ):
            for dc in range(ND):
                nc.tensor.matmul(psb[:, jc:jc + 1],
                                 PT[:, bass.ds(dc * PLEN + jc * 128, 128)],
                                 wb_sb[dc], start=(dc == 0), stop=(dc == ND - 1))
        sb = ptp.tile([128, NJ], F32, tag="sb", name="sb")
        # -2 shift keeps exp(A) well under the fp8e4 max of 240; it cancels
        # exactly in the softmax normalization
        nc.vector.tensor_scalar_add(out=sb, in0=psb, scalar1=-2.0)
        return sb

    def scores_j(jc, PT8, PcT8, sb, exps):
        pss = ps_big.tile([128, 1024], F32, tag="big", name=f"pss{jc}")
        for ic2 in range(2):
            nc.tensor.matmul(pss[:, ts(ic2, 512)],
                             as3(PT8)[:, :, ts(jc, 128)],
                             as3(PcT8)[:, :, ts(ic2, 512)],
                             start=True, stop=True, perf_mode=DRM)
        nc.scalar.activation(out=exps[jc // 2][:, ts(jc % 2, PLEN)], in_=pss,
                             func=AF.Exp, bias=sb[:, jc:jc + 1], scale=1.0)

    def phase_attn(b, P8, exps):
        """denominator + itr numerators, pair-major so early exps unblock."""
        psd = ps_big.tile([128, 1024], F32, tag="big", name="psd")
        pit = [ps_big.tile([128, 1024], F32, tag="big", name=f"pit{dc}")
               for dc in range(ND)]
        for m in range(NPAIR):
            st, sp = (m == 0), (m == NPAIR - 1)
            for ic2 in range(2):
                nc.tensor.matmul(psd[:, ts(ic2, 512)], as3(ones8),
                                 as3(exps[m])[:, :, ts(ic2, 512)],
                                 start=st, stop=sp, perf_mode=DRM)
            for dc in range(ND):
                for ic2 in range(2):
                    nc.tensor.matmul(pit[dc][:, ts(ic2, 512)],
                                     P8[:, 2 * m:2 * m + 2, ts(dc, 128)],
                                     as3(exps[m])[:, :, ts(ic2, 512)],
                                     start=st, stop=sp, perf_mode=DRM)
        recipb = pitr.tile([128, PLEN], F32, tag="recipb", name="recipb")
        nc.vector.reciprocal_approx_fast(out=recipb, in_=psd)
        itrT = [pitr.tile([128, PLEN], BF16, tag=f"it{dc}", name=f"itrT{dc}")
                for dc in range(ND)]
        for dc in range(ND):
            nc.vector.tensor_mul(out=itrT[dc], in0=pit[dc], in1=recipb)
        return itrT

    def mlp_group(dc, wi, catT):
        psm = ps_big.tile([128, 1024], F32, tag="big", name=f"psm{dc}{wi}")
        for pc in range(2):
            for kc in range(4):
                nc.tensor.matmul(
                    psm[:, ts(pc, 512)],
                    w_sb[wi][:, kc, ts(dc, 128)],
                    catT[kc][:, ts(pc, 512)],
                    start=(kc == 0), stop=(kc == 3),
                )
        t = pmlp.tile([128, PLEN], BF16, tag=f"act{wi}", name=f"act{dc}{wi}")
        nc.scalar.activation(out=t, in_=psm, func=AF.Tanh, bias=b_sb[wi][dc],
                             scale=(1.0 if wi == 0 else 0.5))
        return t

    def gate(dc, z_t, t2, t3, Ph, oT):
        # out^T = (t2+1)*(P/2) + 0.5*[(t3+1)*z]; middle product on GpSimd
        sl = ts(dc, PLEN)
        m1 = pmlp.tile([128, PLEN], BF16, tag="m1", name="m1", bufs=2)
        nc.vector.scalar_tensor_tensor(out=m1, in0=t2, scalar=1.0, in1=Ph[:, sl],
                                       op0=ALU.add, op1=ALU.mult)
        m2 = pmlp.tile([128, PLEN], BF16, tag="m2", name="m2", bufs=2)
        nc.vector.scalar_tensor_tensor(out=m2, in0=t3, scalar=1.0, in1=z_t,
                                       op0=ALU.add, op1=ALU.mult)
        nc.vector.scalar_tensor_tensor(out=oT[:, sl], in0=m2, scalar=0.5,
                                       in1=m1, op0=ALU.mult, op1=ALU.add)

    def outpair(b, p2, oT, final=False):
        onat = pout.tile([128, D], BF16, tag=f"on{p2}", name=f"onat{p2}")
        if final:
            # mlp/scores are done: borrow the free big pool so the 8 final
            # transpose pairs don't serialize on a single bank
            pstb = ps_big.tile([128, 1024], F32, tag="big", name="pstf")
            pst = pstb[:, 0:128].bitcast(BF16)
        else:
            pst = ps_t2.tile([128, 256], BF16, tag="pst", name="pst")
        nc.tensor.transpose(pst[:, 0:128],
                            oT[:, bass.ds(0 * PLEN + p2 * 128, 128)], ident_bf)
        nc.tensor.transpose(pst[:, 128:256],
                            oT[:, bass.ds(1 * PLEN + p2 * 128, 128)], ident_bf)
        nc.vector.tensor_copy(out=onat, in_=pst)
        nc.sync.dma_start(out=out[b, ts(p2, 128), :], in_=onat)

    # ---- prologue: batch 0 load + casts + sb + scores, PE kept warm ----
    _consts_early()
    P8, PT = phase_load(0)
    _consts_late()
    # trigger the exp/tanh ACT table load long before the first real exp
    warm_act = const.tile([128, 1], F32, tag="warm_act")
    nc.scalar.activation(out=warm_act, in_=ones_f[:, 0:1], func=AF.Exp,
                         bias=0.0, scale=1.0)
    for k in range(12):
        dummy_mm(k)
    PT8, PcT8 = prep_casts(PT)
    Ph = prep_ph(PT)
    sb = phase_sb(PT)
    exps = [pexp.tile([128, 2 * PLEN], FP8, tag=f"es{m}", name=f"expS{m}")
            for m in range(NPAIR)]
    for jc in range(NJ):
        scores_j(jc, PT8, PcT8, sb, exps)
        dummy_mm(100 + jc)
        dummy_mm(200 + jc)
        dummy_mm(300 + jc)

    # ---- steady loop, software-pipelined ----
    oT_prev = None
    for b in range(B_LOC):
        last = b + 1 >= B_LOC
        if not last:
            P8_n, PT_n = phase_load(b + 1)
        itrT = phase_attn(b, P8, exps)
        if not last:
            sb_n = phase_sb(PT_n)  # fills the PE bubble while DVE normalizes
        if b == 0:
            for k in range(6):
                dummy_mm(400 + k)
        if not last:
            PT8_n, PcT8_n = prep_casts(PT_n)   # DVE, right after recip/norm
            exps_n = [pexp.tile([128, 2 * PLEN], FP8, tag=f"es{m}",
                                name=f"expS{m}") for m in range(NPAIR)]
        catT = [PT[:, 0:PLEN], PT[:, PLEN:2 * PLEN], itrT[0], itrT[1]]
        oT = pmlp.tile([128, 2 * PLEN], BF16, tag="oT", name="oT")

        op = (lambda p2: outpair(b - 1, p2, oT_prev)) if oT_prev is not None \
            else (lambda p2: None)
        z0 = mlp_group(0, 0, catT)
        t2_0 = mlp_group(0, 1, catT)
        op(0)
        t3_0 = mlp_group(0, 2, catT)
        op(1)
        gate(0, z0, t2_0, t3_0, Ph, oT)
        z1 = mlp_group(1, 0, catT)
        if not last:
            scores_j(0, PT8_n, PcT8_n, sb_n, exps_n)
        op(2)
        t2_1 = mlp_group(1, 1, catT)
        if not last:
            scores_j(1, PT8_n, PcT8_n, sb_n, exps_n)
            scores_j(2, PT8_n, PcT8_n, sb_n, exps_n)
        op(3)
        t3_1 = mlp_group(1, 2, catT)
        if not last:
            scores_j(3, PT8_n, PcT8_n, sb_n, exps_n)
            scores_j(4, PT8_n, PcT8_n, sb_n, exps_n)
        op(4)
        gate(1, z1, t2_1, t3_1, Ph, oT)
        if not last:
            for jc in (5, 6, 7):
                scores_j(jc, PT8_n, PcT8_n, sb_n, exps_n)
                op(jc)
            Ph_n = prep_ph(PT_n)
            P8, PT, PT8, PcT8, Ph, exps = P8_n, PT_n, PT8_n, PcT8_n, Ph_n, exps_n
        else:
            for p2 in (5, 6, 7):
                op(p2)
        oT_prev = oT

    for p2 in range(NJ):
        outpair(B_LOC - 1, p2, oT_prev, final=True)


_NC_CACHE = {}


def _build():
    if "nc" in _NC_CACHE:
        return _NC_CACHE["nc"]
    nc = bacc.Bacc("TRN2", target_bir_lowering=False, debug=False,
                   num_devices=N_CORES)
    P_in = nc.dram_tensor("p_in", [B_LOC, PLEN, D], BF16, kind="ExternalInput").ap()
    P8_in = nc.dram_tensor("p8_in", [B_LOC, PLEN, D], FP8, kind="ExternalInput").ap()
    wb_in = nc.dram_tensor("wb", [D], BF16, kind="ExternalInput").ap()
    wc_in = nc.dram_tensor("wc", [D], F32, kind="ExternalInput").ap()
    w_mlp = [nc.dram_tensor(f"w{i}", [2 * D, D], BF16, kind="ExternalInput").ap()
             for i in (1, 2, 3)]
    b_mlp = [nc.dram_tensor(f"b{i}", [D], F32, kind="ExternalInput").ap()
             for i in (1, 2, 3)]
    out = nc.dram_tensor("out", [B_LOC, PLEN, D], BF16, kind="ExternalOutput").ap()

    from contextlib import ExitStack

    with tile.TileContext(nc) as tc, ExitStack() as ctx:
        _emit(ctx, tc, P_in, P8_in, wb_in, wc_in, w_mlp, b_mlp, out)
    nc.compile()
    _NC_CACHE["nc"] = nc
    return nc


def run(inputs, trace=False, tmpdir=None):
    nc = _build()
    bf = ml_dtypes.bfloat16
    e4 = ml_dtypes.float8_e4m3
    Pf = np.ascontiguousarray(np.asarray(inputs["P"], dtype=np.float32))
    P = Pf.astype(bf)
    P8 = Pf.astype(e4)
    w_att = np.asarray(inputs["w_itr_att"], np.float32)
    shared = {
        "wb": np.ascontiguousarray(w_att[D:2 * D]).astype(bf),
        "wc": np.ascontiguousarray(w_att[2 * D:3 * D]),
        "w1": np.ascontiguousarray(np.asarray(inputs["w1"], np.float32)).astype(bf),
        "w2": np.ascontiguousarray(np.asarray(inputs["w2"], np.float32)).astype(bf),
        "w3": np.ascontiguousarray(np.asarray(inputs["w3"], np.float32)).astype(bf),
        "b1": np.ascontiguousarray(np.asarray(inputs["b1"], np.float32)),
        "b2": np.ascontiguousarray(np.asarray(inputs["b2"], np.float32)),
        "b3": np.ascontiguousarray(np.asarray(inputs["b3"], np.float32)),
    }
    in_maps = [
        {"p_in": P[c * B_LOC : (c + 1) * B_LOC],
         "p8_in": P8[c * B_LOC : (c + 1) * B_LOC], **shared}
        for c in range(N_CORES)
    ]
    res = run_bass_kernel_spmd(nc, in_maps, list(range(N_CORES)), trace=trace,
                               tmpdir=tmpdir)
    full = np.concatenate(
        [np.asarray(res.results[c]["out"]).astype(np.float32)
         for c in range(N_CORES)], axis=0)
    return full, res


def kernel(**inputs):
    full, _ = run(inputs)
    return full


# revision 18
# speedup vs baseline: 1.1223x; 1.1223x over previous
"""Trainium2 Bass kernel for the pairwise-score attention + gated MLP encoding.

Computation (per batch element b, p=1024 tokens, d=256 features):
    A[i,j]  = wa.P_i + wb.P_j + (P_i*wc).P_j
    itr     = softmax_j(A) @ P
    cat     = [P, itr]
    z       = tanh(cat@w1+b1); r = sigmoid(cat@w2+b2); f = sigmoid(cat@w3+b3)
    out     = r*P + f*z

Sharding: data-parallel over batch across 8 NeuronCores (4 batch el / core).

v3 kernel structure (per batch element):
  - P shipped bf16 (for P^T via XBAR DMA transposes straight from DRAM) and
    fp8e4 (natural layout, the itr-matmul stationary).  The wa.P_i term is
    constant along the softmax axis j and cancels -> never computed.
  - Score + attention matmuls in fp8e4 DoubleRow (K=256 per pass, fp8 peak):
    S^T = PT8.T @ PcT8 per j-chunk, exp on ACT writes fp8 pair tiles, softmax
    denominator via an all-ones fp8 stationary, itr^T numerator with
    stationary natural-fp8 j-chunk pairs.  sb[j] = P_j.wb via 8 tiny DR
    matvecs off PT8, used as the exp bias (with a -2 overflow-safety shift
    that cancels in the softmax).
  - MLP in bf16, transposed (out^T = (cat@w)^T) so b1/b2/b3 are per-partition
    ACT biases; sigmoid as 0.5+0.5*tanh(x/2) keeps one ACT table set.
  - Gating bf16: out = (t2+1)*(P/2)+0.5*[(t3+1)*z]; the middle product runs
    on GpSimd, the rest on DVE.
  - Output transposed back on the PE in bf16, stored bf16 via the GpSimd DMA
    queue; host casts to f32.
  - The emission interleaves scores(b+1)+exp(b+1) into mlp(b) so the ACT
    queue (14 activates/batch) and the PE stay co-saturated, and out(b-1)
    transpose pairs fill the remaining PE bubbles.  Dummy PE transposes at
    the head keep HAM from idling back to the cold clock.
"""

import sys

if "/opt/trn_rl_repo" not in sys.path:
    sys.path.insert(0, "/opt/trn_rl_repo")

import numpy as np
import ml_dtypes

import concourse.bass as bass
import concourse.mybir as mybir
import concourse.tile as tile
from concourse import bacc
from concourse.bass_utils import run_bass_kernel_spmd
from concourse.masks import make_identity

F32 = mybir.dt.float32
BF16 = mybir.dt.bfloat16
FP8 = mybir.dt.float8e4
AF = mybir.ActivationFunctionType
ALU = mybir.AluOpType
DRM = mybir.MatmulPerfMode.DoubleRow

B, PLEN, D = 32, 1024, 256
N_CORES = 8
B_LOC = B // N_CORES  # batch elements per core

NJ = PLEN // 128  # 8 token chunks of 128
ND = D // 128     # 2 feature chunks of 128
NPAIR = NJ // 2   # 4 token-chunk pairs (fp8 DoubleRow K=256)


def _emit(ctx, tc, P_in, P8_in, wb_in, wc_in, w_mlp, w8_mlp, b_mlp, out):
    nc = tc.nc
    ts = bass.ts

    const = ctx.enter_context(tc.tile_pool(name="const", bufs=1))
    pin = ctx.enter_context(tc.tile_pool(name="pin", bufs=2))
    ptp = ctx.enter_context(tc.tile_pool(name="ptp", bufs=2))
    pexp = ctx.enter_context(tc.tile_pool(name="pexp", bufs=2))
    pitr = ctx.enter_context(tc.tile_pool(name="pitr", bufs=2))
    pmlp = ctx.enter_context(tc.tile_pool(name="pmlp", bufs=2))
    pout = ctx.enter_context(tc.tile_pool(name="pout", bufs=1))
    # PSUM is 8 banks: ps_big 3x2 + pst 2
    ps_big = ctx.enter_context(tc.tile_pool(name="ps_big", bufs=3, space="PSUM"))
    ps_t2 = ctx.enter_context(tc.tile_pool(name="ps_t2", bufs=2, space="PSUM"))

    as3 = lambda ap: ap.rearrange("p (c x) -> p c x", c=2)

    # ---- constants (once per core) ----
    ident = const.tile([128, 128], F32)
    make_identity(nc, ident)
    ident_bf = const.tile([128, 128], BF16)
    nc.vector.tensor_copy(out=ident_bf, in_=ident)
    ones_f = const.tile([128, 256], F32)
    nc.vector.memset(ones_f, 1.0)
    ones8 = const.tile([128, 256], FP8)  # as3 -> [128, 2, 128] DR stationary
    nc.vector.tensor_copy(out=ones8, in_=ones_f)
    neg2 = const.tile([128, 1], F32)
    nc.vector.memset(neg2, -2.0)

    # const weight DMAs are emitted inside the prologue on the sync queue,
    # ordered by first use (see _prologue_consts)
    wb_sb, wc_sb, w_sb, w8_sb, b_sb = [], [], [], [], []

    def _consts_early():
        # wb/wc: operands of the fused PcT8 = PT*wc + wb cast
        for dc in range(ND):
            t = const.tile([128, 1], F32, tag=f"wb{dc}")
            nc.sync.dma_start(out=t, in_=wb_in[bass.ds(dc * 128, 128)].unsqueeze(1))
            wb_sb.append(t)
        for dc in range(ND):
            t = const.tile([128, 1], F32, tag=f"wc{dc}")
            nc.sync.dma_start(out=t, in_=wc_in[bass.ds(dc * 128, 128)].unsqueeze(1))
            wc_sb.append(t)

    def _consts_late():
        # MLP weights: P-part (rows 0..255) bf16 [128, 2(kc), 256];
        # itr-part (rows 256..511) fp8 pair layout [128, 2(c), 256]
        for wi in range(3):
            wt = const.tile([128, 2, D], BF16, tag=f"w{wi}")
            nc.sync.dma_start(
                out=wt,
                in_=w_mlp[wi][0:D, :].rearrange("(kc k) d -> k kc d", k=128))
            w_sb.append(wt)
        for wi in range(3):
            wt = const.tile([128, 2, D], FP8, tag=f"w8{wi}")
            nc.sync.dma_start(
                out=wt,
                in_=w8_mlp[wi].rearrange("(c k) d -> k c d", k=128))
            w8_sb.append(wt)
        # biases, per dout-chunk [128,1]; for r/f (sigmoid-via-tanh): b/2
        for wi in range(3):
            chunks = []
            for dc in range(ND):
                bt = const.tile([128, 1], F32, tag=f"b{wi}{dc}")
                nc.sync.dma_start(out=bt,
                                  in_=b_mlp[wi][bass.ds(dc * 128, 128)].unsqueeze(1))
                if wi > 0:
                    bh = const.tile([128, 1], F32, tag=f"bh{wi}{dc}")
                    nc.scalar.mul(out=bh, in_=bt, mul=0.5)
                    bt = bh
                chunks.append(bt)
            b_sb.append(chunks)

    # ---- helpers ----
    def dummy_mm(k):
        """Keep-warm PE transpose with no data dependencies."""
        t = ps_t2.tile([128, 256], BF16, tag="pst", name=f"dum{k}")
        nc.tensor.transpose(t[:, 0:128], ident_bf, ident_bf)

    def phase_load(b):
        """fp8 natural tile + XBAR bf16 transposes, all on the sync queue."""
        P8 = pin.tile([128, NJ, D], FP8, tag="p8", name="P8")
        nc.sync.dma_start(out=P8,
                          in_=P8_in[b].rearrange("(jc k) d -> k jc d", k=128))
        # bufs=3: PT is written at iter-(b-1) start and read through iter b's
        # mlp, so a 2-deep rotation would stall the next XBAR on mlp(b-1)
        PT = ptp.tile([128, 2 * PLEN], BF16, tag="pt", name="PT", bufs=3)
        for dc in range(ND):
            nc.sync.dma_start_transpose(out=PT[:, ts(dc, PLEN)],
                                        in_=P_in[b, :, ts(dc, 128)])
        return P8, PT

    def prep_casts(PT):
        """PT8 + PcT8 on DVE (early; feeds sb matvecs and scores)."""
        PT8 = ptp.tile([128, 2 * PLEN], FP8, tag="pt8", name="PT8")
        nc.vector.tensor_copy(out=PT8, in_=PT)
        # PcT8 = PT*wc + wb: the wb term turns into the sb_j softmax bias
        # inside the scores matmul (sum_d PT[d,j]*wb_d = P_j . wb)
        PcT8 = ptp.tile([128, 2 * PLEN], FP8, tag="pct8", name="PcT8")
        for dc in range(ND):
            nc.vector.tensor_scalar(out=PcT8[:, ts(dc, PLEN)],
                                    in0=PT[:, ts(dc, PLEN)],
                                    scalar1=wc_sb[dc], scalar2=wb_sb[dc],
                                    op0=ALU.mult, op1=ALU.add)
        return PT8, PcT8

    def prep_ph(PT):
        Ph = ptp.tile([128, 2 * PLEN], BF16, tag="ph", name="Ph")
        nc.vector.tensor_scalar_mul(out=Ph, in0=PT, scalar1=0.5)
        return Ph

    def scores_j(jc, PT8, PcT8, exps):
        pss = ps_big.tile([128, 1024], F32, tag="big", name=f"pss{jc}")
        for ic2 in range(2):
            nc.tensor.matmul(pss[:, ts(ic2, 512)],
                             as3(PT8)[:, :, ts(jc, 128)],
                             as3(PcT8)[:, :, ts(ic2, 512)],
                             start=True, stop=True, perf_mode=DRM)
        # -2 shift keeps exp(A) well under the fp8e4 max of 240; it cancels
        # exactly in the softmax normalization
        nc.scalar.activation(out=exps[jc // 2][:, ts(jc % 2, PLEN)], in_=pss,
                             func=AF.Exp, bias=neg2, scale=1.0)

    def phase_attn(b, P8, exps):
        """denominator + itr numerators, pair-major so early exps unblock."""
        psd = ps_big.tile([128, 1024], F32, tag="big", name="psd")
        pit = [ps_big.tile([128, 1024], F32, tag="big", name=f"pit{dc}")
               for dc in range(ND)]
        for m in range(NPAIR):
            st, sp = (m == 0), (m == NPAIR - 1)
            for ic2 in range(2):
                nc.tensor.matmul(psd[:, ts(ic2, 512)], as3(ones8),
                                 as3(exps[m])[:, :, ts(ic2, 512)],
                                 start=st, stop=sp, perf_mode=DRM)
            for dc in range(ND):
                for ic2 in range(2):
                    nc.tensor.matmul(pit[dc][:, ts(ic2, 512)],
                                     P8[:, 2 * m:2 * m + 2, ts(dc, 128)],
                                     as3(exps[m])[:, :, ts(ic2, 512)],
                                     start=st, stop=sp, perf_mode=DRM)
        recipb = pitr.tile([128, PLEN], F32, tag="recipb", name="recipb")
        nc.vector.reciprocal_approx_fast(out=recipb, in_=psd)
        itr8 = pitr.tile([128, 2 * PLEN], FP8, tag="it8", name="itr8")
        for dc in range(ND):
            nc.vector.tensor_mul(out=itr8[:, ts(dc, PLEN)], in0=pit[dc],
                                 in1=recipb)
        return itr8

    def mlp_group(dc, wi, PT, itr8):
        psm = ps_big.tile([128, 1024], F32, tag="big", name=f"psm{dc}{wi}")
        for pc in range(2):
            for kc in range(2):
                nc.tensor.matmul(
                    psm[:, ts(pc, 512)],
                    w_sb[wi][:, kc, ts(dc, 128)],
                    PT[:, bass.ds(kc * PLEN + pc * 512, 512)],
                    start=(kc == 0), stop=False,
                )
            nc.tensor.matmul(
                psm[:, ts(pc, 512)],
                w8_sb[wi][:, :, ts(dc, 128)],
                as3(itr8)[:, :, ts(pc, 512)],
                start=False, stop=True, perf_mode=DRM,
            )
        t = pmlp.tile([128, PLEN], BF16, tag=f"act{wi}", name=f"act{dc}{wi}")
        nc.scalar.activation(out=t, in_=psm, func=AF.Tanh, bias=b_sb[wi][dc],
                             scale=(1.0 if wi == 0 else 0.5))
        return t

    def gate(dc, z_t, t2, t3, Ph, oT):
        # out^T = (t2+1)*(P/2) + 0.5*[(t3+1)*z]; middle product on GpSimd
        sl = ts(dc, PLEN)
        m1 = pmlp.tile([128, PLEN], BF16, tag="m1", name="m1", bufs=2)
        nc.vector.scalar_tensor_tensor(out=m1, in0=t2, scalar=1.0, in1=Ph[:, sl],
                                       op0=ALU.add, op1=ALU.mult)
        m2 = pmlp.tile([128, PLEN], BF16, tag="m2", name="m2", bufs=2)
        nc.vector.scalar_tensor_tensor(out=m2, in0=t3, scalar=1.0, in1=z_t,
                                       op0=ALU.add, op1=ALU.mult)
        nc.vector.scalar_tensor_tensor(out=oT[:, sl], in0=m2, scalar=0.5,
                                       in1=m1, op0=ALU.mult, op1=ALU.add)

    def outpair(b, p2, oT, final=False):
        onat = pout.tile([128, D], BF16, tag=f"on{p2}", name=f"onat{p2}")
        if final:
            # mlp/scores are done: borrow the free big pool so the 8 final
            # transpose pairs don't serialize on a single bank
            pstb = ps_big.tile([128, 1024], F32, tag="big", name="pstf")
            pst = pstb[:, 0:128].bitcast(BF16)
        else:
            pst = ps_t2.tile([128, 256], BF16, tag="pst", name="pst")
        nc.tensor.transpose(pst[:, 0:128],
                            oT[:, bass.ds(0 * PLEN + p2 * 128, 128)], ident_bf)
        nc.tensor.transpose(pst[:, 128:256],
                            oT[:, bass.ds(1 * PLEN + p2 * 128, 128)], ident_bf)
        nc.vector.tensor_copy(out=onat, in_=pst)
        nc.sync.dma_start(out=out[b, ts(p2, 128), :], in_=onat)

    # ---- prologue: batch 0 load + casts + sb + scores, PE kept warm ----
    _consts_early()
    P8, PT = phase_load(0)
    _consts_late()
    # trigger the exp/tanh ACT table load long before the first real exp
    warm_act = const.tile([128, 1], F32, tag="warm_act")
    nc.scalar.activation(out=warm_act, in_=ones_f[:, 0:1], func=AF.Exp,
                         bias=0.0, scale=1.0)
    for k in range(12):
        dummy_mm(k)
    PT8, PcT8 = prep_casts(PT)
    Ph = prep_ph(PT)
    exps = [pexp.tile([128, 2 * PLEN], FP8, tag=f"es{m}", name=f"expS{m}")
            for m in range(NPAIR)]
    for jc in range(NJ):
        scores_j(jc, PT8, PcT8, exps)
        dummy_mm(100 + jc)
        dummy_mm(200 + jc)
        dummy_mm(300 + jc)

    # ---- steady loop, software-pipelined ----
    oT_prev = None
    for b in range(B_LOC):
        last = b + 1 >= B_LOC
        if not last:
            P8_n, PT_n = phase_load(b + 1)
        itr8 = phase_attn(b, P8, exps)
        if b == 0:
            for k in range(6):
                dummy_mm(400 + k)
        if not last:
            PT8_n, PcT8_n = prep_casts(PT_n)   # DVE, right after recip/norm
            exps_n = [pexp.tile([128, 2 * PLEN], FP8, tag=f"es{m}",
                                name=f"expS{m}") for m in range(NPAIR)]
        oT = pmlp.tile([128, 2 * PLEN], BF16, tag="oT", name="oT")

        op = (lambda p2: outpair(b - 1, p2, oT_prev)) if oT_prev is not None \
            else (lambda p2: None)
        z0 = mlp_group(0, 0, PT, itr8)
        t2_0 = mlp_group(0, 1, PT, itr8)
        op(0)
        t3_0 = mlp_group(0, 2, PT, itr8)
        op(1)
        gate(0, z0, t2_0, t3_0, Ph, oT)
        z1 = mlp_group(1, 0, PT, itr8)
        if not last:
            scores_j(0, PT8_n, PcT8_n, exps_n)
        op(2)
        t2_1 = mlp_group(1, 1, PT, itr8)
        if not last:
            scores_j(1, PT8_n, PcT8_n, exps_n)
            scores_j(2, PT8_n, PcT8_n, exps_n)
        op(3)
        t3_1 = mlp_group(1, 2, PT, itr8)
        if not last:
            scores_j(3, PT8_n, PcT8_n, exps_n)
            scores_j(4, PT8_n, PcT8_n, exps_n)
        op(4)
        gate(1, z1, t2_1, t3_1, Ph, oT)
        if not last:
            for jc in (5, 6, 7):
                scores_j(jc, PT8_n, PcT8_n, exps_n)
                op(jc)
            Ph_n = prep_ph(PT_n)
            P8, PT, PT8, PcT8, Ph, exps = P8_n, PT_n, PT8_n, PcT8_n, Ph_n, exps_n
        else:
            for p2 in (5, 6, 7):
                op(p2)
        oT_prev = oT

    for p2 in range(NJ):
        outpair(B_LOC - 1, p2, oT_prev, final=True)


_NC_CACHE = {}


def _build():
    if "nc" in _NC_CACHE:
        return _NC_CACHE["nc"]
    nc = bacc.Bacc("TRN2", target_bir_lowering=False, debug=False,
                   num_devices=N_CORES)
    P_in = nc.dram_tensor("p_in", [B_LOC, PLEN, D], BF16, kind="ExternalInput").ap()
    P8_in = nc.dram_tensor("p8_in", [B_LOC, PLEN, D], FP8, kind="ExternalInput").ap()
    wb_in = nc.dram_tensor("wb", [D], F32, kind="ExternalInput").ap()
    wc_in = nc.dram_tensor("wc", [D], F32, kind="ExternalInput").ap()
    w_mlp = [nc.dram_tensor(f"w{i}", [2 * D, D], BF16, kind="ExternalInput").ap()
             for i in (1, 2, 3)]
    w8_mlp = [nc.dram_tensor(f"w8{i}", [D, D], FP8, kind="ExternalInput").ap()
              for i in (1, 2, 3)]
    b_mlp = [nc.dram_tensor(f"b{i}", [D], F32, kind="ExternalInput").ap()
             for i in (1, 2, 3)]
    out = nc.dram_tensor("out", [B_LOC, PLEN, D], BF16, kind="ExternalOutput").ap()

    from contextlib import ExitStack

    with tile.TileContext(nc) as tc, ExitStack() as ctx:
        _emit(ctx, tc, P_in, P8_in, wb_in, wc_in, w_mlp, w8_mlp, b_mlp, out)
    nc.compile()
    _NC_CACHE["nc"] = nc
    return nc


def run(inputs, trace=False, tmpdir=None):
    nc = _build()
    bf = ml_dtypes.bfloat16
    e4 = ml_dtypes.float8_e4m3
    Pf = np.ascontiguousarray(np.asarray(inputs["P"], dtype=np.float32))
    P = Pf.astype(bf)
    P8 = Pf.astype(e4)
    w_att = np.asarray(inputs["w_itr_att"], np.float32)
    shared = {
        "wb": np.ascontiguousarray(w_att[D:2 * D]),
        "wc": np.ascontiguousarray(w_att[2 * D:3 * D]),
        "w1": np.ascontiguousarray(np.asarray(inputs["w1"], np.float32)).astype(bf),
        "w2": np.ascontiguousarray(np.asarray(inputs["w2"], np.float32)).astype(bf),
        "w3": np.ascontiguousarray(np.asarray(inputs["w3"], np.float32)).astype(bf),
        "w81": np.ascontiguousarray(np.asarray(inputs["w1"], np.float32)[D:]).astype(e4),
        "w82": np.ascontiguousarray(np.asarray(inputs["w2"], np.float32)[D:]).astype(e4),
        "w83": np.ascontiguousarray(np.asarray(inputs["w3"], np.float32)[D:]).astype(e4),
        "b1": np.ascontiguousarray(np.asarray(inputs["b1"], np.float32)),
        "b2": np.ascontiguousarray(np.asarray(inputs["b2"], np.float32)),
        "b3": np.ascontiguousarray(np.asarray(inputs["b3"], np.float32)),
    }
    in_maps = [
        {"p_in": P[c * B_LOC : (c + 1) * B_LOC],
         "p8_in": P8[c * B_LOC : (c + 1) * B_LOC], **shared}
        for c in range(N_CORES)
    ]
    res = run_bass_kernel_spmd(nc, in_maps, list(range(N_CORES)), trace=trace,
                               tmpdir=tmpdir)
    full = np.concatenate(
        [np.asarray(res.results[c]["out"]).astype(np.float32)
         for c in range(N_CORES)], axis=0)
    return full, res


def kernel(**inputs):
    full, _ = run(inputs)
    return full


# revision 19
# speedup vs baseline: 1.1368x; 1.0130x over previous
"""Trainium2 Bass kernel for the pairwise-score attention + gated MLP encoding.

Computation (per batch element b, p=1024 tokens, d=256 features):
    A[i,j]  = wa.P_i + wb.P_j + (P_i*wc).P_j
    itr     = softmax_j(A) @ P
    cat     = [P, itr]
    z       = tanh(cat@w1+b1); r = sigmoid(cat@w2+b2); f = sigmoid(cat@w3+b3)
    out     = r*P + f*z

Sharding: data-parallel over batch across 8 NeuronCores (4 batch el / core).

v3 kernel structure (per batch element):
  - P shipped bf16 (for P^T via XBAR DMA transposes straight from DRAM) and
    fp8e4 (natural layout, the itr-matmul stationary).  The wa.P_i term is
    constant along the softmax axis j and cancels -> never computed.
  - Score + attention matmuls in fp8e4 DoubleRow (K=256 per pass, fp8 peak):
    S^T = PT8.T @ PcT8 per j-chunk, exp on ACT writes fp8 pair tiles, softmax
    denominator via an all-ones fp8 stationary, itr^T numerator with
    stationary natural-fp8 j-chunk pairs.  sb[j] = P_j.wb via 8 tiny DR
    matvecs off PT8, used as the exp bias (with a -2 overflow-safety shift
    that cancels in the softmax).
  - MLP in bf16, transposed (out^T = (cat@w)^T) so b1/b2/b3 are per-partition
    ACT biases; sigmoid as 0.5+0.5*tanh(x/2) keeps one ACT table set.
  - Gating bf16: out = (t2+1)*(P/2)+0.5*[(t3+1)*z]; the middle product runs
    on GpSimd, the rest on DVE.
  - Output transposed back on the PE in bf16, stored bf16 via the GpSimd DMA
    queue; host casts to f32.
  - The emission interleaves scores(b+1)+exp(b+1) into mlp(b) so the ACT
    queue (14 activates/batch) and the PE stay co-saturated, and out(b-1)
    transpose pairs fill the remaining PE bubbles.  Dummy PE transposes at
    the head keep HAM from idling back to the cold clock.
"""

import sys

if "/opt/trn_rl_repo" not in sys.path:
    sys.path.insert(0, "/opt/trn_rl_repo")

import numpy as np
import ml_dtypes

import concourse.bass as bass
import concourse.mybir as mybir
import concourse.tile as tile
from concourse import bacc
from concourse.bass_utils import run_bass_kernel_spmd
from concourse.masks import make_identity

F32 = mybir.dt.float32
BF16 = mybir.dt.bfloat16
FP8 = mybir.dt.float8e4
AF = mybir.ActivationFunctionType
ALU = mybir.AluOpType
DRM = mybir.MatmulPerfMode.DoubleRow

B, PLEN, D = 32, 1024, 256
N_CORES = 8
B_LOC = B // N_CORES  # batch elements per core

NJ = PLEN // 128  # 8 token chunks of 128
ND = D // 128     # 2 feature chunks of 128
NPAIR = NJ // 2   # 4 token-chunk pairs (fp8 DoubleRow K=256)


def _emit(ctx, tc, P_in, P8_in, wb_in, wc_in, w_mlp, w8_mlp, b_mlp, out):
    nc = tc.nc
    ts = bass.ts

    const = ctx.enter_context(tc.tile_pool(name="const", bufs=1))
    pin = ctx.enter_context(tc.tile_pool(name="pin", bufs=2))
    ptp = ctx.enter_context(tc.tile_pool(name="ptp", bufs=2))
    pexp = ctx.enter_context(tc.tile_pool(name="pexp", bufs=2))
    pitr = ctx.enter_context(tc.tile_pool(name="pitr", bufs=2))
    pmlp = ctx.enter_context(tc.tile_pool(name="pmlp", bufs=2))
    pout = ctx.enter_context(tc.tile_pool(name="pout", bufs=1))
    # PSUM is 8 banks: ps_big 3x2 + pst 2
    ps_big = ctx.enter_context(tc.tile_pool(name="ps_big", bufs=3, space="PSUM"))
    ps_t2 = ctx.enter_context(tc.tile_pool(name="ps_t2", bufs=2, space="PSUM"))

    as3 = lambda ap: ap.rearrange("p (c x) -> p c x", c=2)

    # ---- constants (once per core) ----
    ident = const.tile([128, 128], F32)
    make_identity(nc, ident)
    ident_bf = const.tile([128, 128], BF16)
    nc.vector.tensor_copy(out=ident_bf, in_=ident)
    ones_f = const.tile([128, 256], F32)
    nc.vector.memset(ones_f, 1.0)
    ones8 = const.tile([128, 256], FP8)  # as3 -> [128, 2, 128] DR stationary
    nc.vector.tensor_copy(out=ones8, in_=ones_f)
    neg2 = const.tile([128, 1], F32)
    nc.vector.memset(neg2, -2.0)

    # const weight DMAs are emitted inside the prologue on the sync queue,
    # ordered by first use (see _prologue_consts)
    wb_sb, wc_sb, w_sb, w8_sb, b_sb = [], [], [], [], []

    def _consts_early():
        # wb/wc: operands of the fused PcT8 = PT*wc + wb cast
        for dc in range(ND):
            t = const.tile([128, 1], F32, tag=f"wb{dc}")
            nc.sync.dma_start(out=t, in_=wb_in[bass.ds(dc * 128, 128)].unsqueeze(1))
            wb_sb.append(t)
        for dc in range(ND):
            t = const.tile([128, 1], F32, tag=f"wc{dc}")
            nc.sync.dma_start(out=t, in_=wc_in[bass.ds(dc * 128, 128)].unsqueeze(1))
            wc_sb.append(t)

    def _consts_late():
        # MLP weights: P-part (rows 0..255) bf16 [128, 2(kc), 256];
        # itr-part (rows 256..511) fp8 pair layout [128, 2(c), 256]
        for wi in range(3):
            wt = const.tile([128, 2, D], BF16, tag=f"w{wi}")
            nc.sync.dma_start(
                out=wt,
                in_=w_mlp[wi][0:D, :].rearrange("(kc k) d -> k kc d", k=128))
            w_sb.append(wt)
        for wi in range(3):
            wt = const.tile([128, 2, D], FP8, tag=f"w8{wi}")
            nc.sync.dma_start(
                out=wt,
                in_=w8_mlp[wi].rearrange("(c k) d -> k c d", k=128))
            w8_sb.append(wt)
        # biases, per dout-chunk [128,1]; for r/f (sigmoid-via-tanh): b/2
        for wi in range(3):
            chunks = []
            for dc in range(ND):
                bt = const.tile([128, 1], F32, tag=f"b{wi}{dc}")
                nc.sync.dma_start(out=bt,
                                  in_=b_mlp[wi][bass.ds(dc * 128, 128)].unsqueeze(1))
                if wi > 0:
                    bh = const.tile([128, 1], F32, tag=f"bh{wi}{dc}")
                    nc.scalar.mul(out=bh, in_=bt, mul=0.5)
                    bt = bh
                chunks.append(bt)
            b_sb.append(chunks)

    # ---- helpers ----
    def dummy_mm(k):
        """Keep-warm PE transpose with no data dependencies."""
        t = ps_t2.tile([128, 256], BF16, tag="pst", name=f"dum{k}")
        nc.tensor.transpose(t[:, 0:128], ident_bf, ident_bf)

    def phase_load(b):
        """fp8 natural tile + XBAR bf16 transposes, all on the sync queue."""
        P8 = pin.tile([128, NJ, D], FP8, tag="p8", name="P8")
        nc.sync.dma_start(out=P8,
                          in_=P8_in[b].rearrange("(jc k) d -> k jc d", k=128))
        # bufs=3: PT is written at iter-(b-1) start and read through iter b's
        # mlp, so a 2-deep rotation would stall the next XBAR on mlp(b-1)
        PT = ptp.tile([128, 2 * PLEN], BF16, tag="pt", name="PT", bufs=3)
        for dc in range(ND):
            nc.sync.dma_start_transpose(out=PT[:, ts(dc, PLEN)],
                                        in_=P_in[b, :, ts(dc, 128)])
        return P8, PT

    def prep_casts(PT):
        """PT8 + PcT8 on DVE (early; feeds sb matvecs and scores)."""
        PT8 = ptp.tile([128, 2 * PLEN], FP8, tag="pt8", name="PT8")
        nc.vector.tensor_copy(out=PT8, in_=PT)
        # PcT8 = PT*wc + wb: the wb term turns into the sb_j softmax bias
        # inside the scores matmul (sum_d PT[d,j]*wb_d = P_j . wb)
        PcT8 = ptp.tile([128, 2 * PLEN], FP8, tag="pct8", name="PcT8")
        for dc in range(ND):
            nc.vector.tensor_scalar(out=PcT8[:, ts(dc, PLEN)],
                                    in0=PT[:, ts(dc, PLEN)],
                                    scalar1=wc_sb[dc], scalar2=wb_sb[dc],
                                    op0=ALU.mult, op1=ALU.add)
        return PT8, PcT8

    def prep_ph(PT):
        Ph = ptp.tile([128, 2 * PLEN], BF16, tag="ph", name="Ph")
        nc.vector.tensor_scalar_mul(out=Ph, in0=PT, scalar1=0.5)
        return Ph

    def scores_j(jc, PT8, PcT8, exps):
        pss = ps_big.tile([128, 1024], F32, tag="big", name=f"pss{jc}")
        for ic2 in range(2):
            nc.tensor.matmul(pss[:, ts(ic2, 512)],
                             as3(PT8)[:, :, ts(jc, 128)],
                             as3(PcT8)[:, :, ts(ic2, 512)],
                             start=True, stop=True, perf_mode=DRM)
        # -2 shift keeps exp(A) well under the fp8e4 max of 240; it cancels
        # exactly in the softmax normalization
        nc.scalar.activation(out=exps[jc // 2][:, ts(jc % 2, PLEN)], in_=pss,
                             func=AF.Exp, bias=neg2, scale=1.0)

    def phase_attn(b, P8, exps):
        """denominator + itr numerators, pair-major so early exps unblock."""
        psd = ps_big.tile([128, 1024], F32, tag="big", name="psd")
        pit = [ps_big.tile([128, 1024], F32, tag="big", name=f"pit{dc}")
               for dc in range(ND)]
        for m in range(NPAIR):
            st, sp = (m == 0), (m == NPAIR - 1)
            for ic2 in range(2):
                nc.tensor.matmul(psd[:, ts(ic2, 512)], as3(ones8),
                                 as3(exps[m])[:, :, ts(ic2, 512)],
                                 start=st, stop=sp, perf_mode=DRM)
            for dc in range(ND):
                for ic2 in range(2):
                    nc.tensor.matmul(pit[dc][:, ts(ic2, 512)],
                                     P8[:, 2 * m:2 * m + 2, ts(dc, 128)],
                                     as3(exps[m])[:, :, ts(ic2, 512)],
                                     start=st, stop=sp, perf_mode=DRM)
        recipb = pitr.tile([128, PLEN], F32, tag="recipb", name="recipb")
        nc.vector.reciprocal_approx_fast(out=recipb, in_=psd)
        itr8 = pitr.tile([128, 2 * PLEN], FP8, tag="it8", name="itr8")
        for dc in range(ND):
            nc.vector.tensor_mul(out=itr8[:, ts(dc, PLEN)], in0=pit[dc],
                                 in1=recipb)
        return itr8

    def mlp_group(dc, wi, PT, itr8):
        psm = ps_big.tile([128, 1024], F32, tag="big", name=f"psm{dc}{wi}")
        for pc in range(2):
            for kc in range(2):
                nc.tensor.matmul(
                    psm[:, ts(pc, 512)],
                    w_sb[wi][:, kc, ts(dc, 128)],
                    PT[:, bass.ds(kc * PLEN + pc * 512, 512)],
                    start=(kc == 0), stop=False,
                )
            nc.tensor.matmul(
                psm[:, ts(pc, 512)],
                w8_sb[wi][:, :, ts(dc, 128)],
                as3(itr8)[:, :, ts(pc, 512)],
                start=False, stop=True, perf_mode=DRM,
            )
        t = pmlp.tile([128, PLEN], BF16, tag=f"act{wi}", name=f"act{dc}{wi}")
        nc.scalar.activation(out=t, in_=psm, func=AF.Tanh, bias=b_sb[wi][dc],
                             scale=(1.0 if wi == 0 else 0.5))
        return t

    def gate(dc, z_t, t2, t3, Ph, oT, halves=False):
        # out^T = (t2+1)*(P/2) + 0.5*[(t3+1)*z]
        m1 = pmlp.tile([128, PLEN], BF16, tag="m1", name="m1", bufs=2)
        m2 = pmlp.tile([128, PLEN], BF16, tag="m2", name="m2", bufs=2)
        for pc in (range(2) if halves else (slice(None),)):
            sl = ts(pc, 512) if halves else slice(0, PLEN)
            osl = (bass.ds(dc * PLEN + pc * 512, 512) if halves
                   else ts(dc, PLEN))
            nc.vector.scalar_tensor_tensor(out=m1[:, sl], in0=t2[:, sl],
                                           scalar=1.0, in1=Ph[:, osl],
                                           op0=ALU.add, op1=ALU.mult)
            nc.vector.scalar_tensor_tensor(out=m2[:, sl], in0=t3[:, sl],
                                           scalar=1.0, in1=z_t[:, sl],
                                           op0=ALU.add, op1=ALU.mult)
            nc.vector.scalar_tensor_tensor(out=oT[:, osl], in0=m2[:, sl],
                                           scalar=0.5, in1=m1[:, sl],
                                           op0=ALU.mult, op1=ALU.add)

    def outpair(b, p2, oT, final=False):
        onat = pout.tile([128, D], BF16, tag=f"on{p2}", name=f"onat{p2}")
        if final:
            # mlp/scores are done: borrow the free big pool so the 8 final
            # transpose pairs don't serialize on a single bank
            pstb = ps_big.tile([128, 1024], F32, tag="big", name="pstf")
            pst = pstb[:, 0:128].bitcast(BF16)
        else:
            pst = ps_t2.tile([128, 256], BF16, tag="pst", name="pst")
        nc.tensor.transpose(pst[:, 0:128],
                            oT[:, bass.ds(0 * PLEN + p2 * 128, 128)], ident_bf)
        nc.tensor.transpose(pst[:, 128:256],
                            oT[:, bass.ds(1 * PLEN + p2 * 128, 128)], ident_bf)
        nc.vector.tensor_copy(out=onat, in_=pst)
        nc.sync.dma_start(out=out[b, ts(p2, 128), :], in_=onat)

    # ---- prologue: batch 0 load + casts + scores, PE kept warm ----
    _consts_early()
    # XBARs first (they gate the whole scores chain), then p8 and weights
    PT = ptp.tile([128, 2 * PLEN], BF16, tag="pt", name="PT", bufs=3)
    for dc in range(ND):
        nc.sync.dma_start_transpose(out=PT[:, ts(dc, PLEN)],
                                    in_=P_in[0, :, ts(dc, 128)])
    P8 = pin.tile([128, NJ, D], FP8, tag="p8", name="P8")
    nc.sync.dma_start(out=P8,
                      in_=P8_in[0].rearrange("(jc k) d -> k jc d", k=128))
    _consts_late()
    # trigger the exp/tanh ACT table load long before the first real exp
    warm_act = const.tile([128, 1], F32, tag="warm_act")
    nc.scalar.activation(out=warm_act, in_=ones_f[:, 0:1], func=AF.Exp,
                         bias=0.0, scale=1.0)
    for k in range(12):
        dummy_mm(k)
    PT8, PcT8 = prep_casts(PT)
    Ph = prep_ph(PT)
    exps = [pexp.tile([128, 2 * PLEN], FP8, tag=f"es{m}", name=f"expS{m}")
            for m in range(NPAIR)]
    for jc in range(NJ):
        scores_j(jc, PT8, PcT8, exps)
        dummy_mm(100 + jc)
        dummy_mm(200 + jc)
        dummy_mm(300 + jc)

    # ---- steady loop, software-pipelined ----
    oT_prev = None
    for b in range(B_LOC):
        last = b + 1 >= B_LOC
        if not last:
            P8_n, PT_n = phase_load(b + 1)
        itr8 = phase_attn(b, P8, exps)
        if b == 0:
            for k in range(6):
                dummy_mm(400 + k)
        if not last:
            PT8_n, PcT8_n = prep_casts(PT_n)   # DVE, right after recip/norm
            exps_n = [pexp.tile([128, 2 * PLEN], FP8, tag=f"es{m}",
                                name=f"expS{m}") for m in range(NPAIR)]
        oT = pmlp.tile([128, 2 * PLEN], BF16, tag="oT", name="oT")

        op = (lambda p2: outpair(b - 1, p2, oT_prev)) if oT_prev is not None \
            else (lambda p2: None)
        # pairs 0/1 fill the PE bubble while DVE computes recip+normalize
        op(0)
        op(1)
        z0 = mlp_group(0, 0, PT, itr8)
        t2_0 = mlp_group(0, 1, PT, itr8)
        t3_0 = mlp_group(0, 2, PT, itr8)
        gate(0, z0, t2_0, t3_0, Ph, oT)
        z1 = mlp_group(1, 0, PT, itr8)
        if not last:
            scores_j(0, PT8_n, PcT8_n, exps_n)
        op(2)
        t2_1 = mlp_group(1, 1, PT, itr8)
        if not last:
            scores_j(1, PT8_n, PcT8_n, exps_n)
            scores_j(2, PT8_n, PcT8_n, exps_n)
        op(3)
        t3_1 = mlp_group(1, 2, PT, itr8)
        if not last:
            scores_j(3, PT8_n, PcT8_n, exps_n)
            scores_j(4, PT8_n, PcT8_n, exps_n)
        op(4)
        gate(1, z1, t2_1, t3_1, Ph, oT, halves=last)
        if not last:
            for jc in (5, 6, 7):
                scores_j(jc, PT8_n, PcT8_n, exps_n)
                op(jc)
            Ph_n = prep_ph(PT_n)
            P8, PT, PT8, PcT8, Ph, exps = P8_n, PT_n, PT8_n, PcT8_n, Ph_n, exps_n
        else:
            for p2 in (5, 6, 7):
                op(p2)
        oT_prev = oT

    for p2 in range(NJ):
        outpair(B_LOC - 1, p2, oT_prev, final=True)


_NC_CACHE = {}


def _build():
    if "nc" in _NC_CACHE:
        return _NC_CACHE["nc"]
    nc = bacc.Bacc("TRN2", target_bir_lowering=False, debug=False,
                   num_devices=N_CORES)
    P_in = nc.dram_tensor("p_in", [B_LOC, PLEN, D], BF16, kind="ExternalInput").ap()
    P8_in = nc.dram_tensor("p8_in", [B_LOC, PLEN, D], FP8, kind="ExternalInput").ap()
    wb_in = nc.dram_tensor("wb", [D], F32, kind="ExternalInput").ap()
    wc_in = nc.dram_tensor("wc", [D], F32, kind="ExternalInput").ap()
    w_mlp = [nc.dram_tensor(f"w{i}", [2 * D, D], BF16, kind="ExternalInput").ap()
             for i in (1, 2, 3)]
    w8_mlp = [nc.dram_tensor(f"w8{i}", [D, D], FP8, kind="ExternalInput").ap()
              for i in (1, 2, 3)]
    b_mlp = [nc.dram_tensor(f"b{i}", [D], F32, kind="ExternalInput").ap()
             for i in (1, 2, 3)]
    out = nc.dram_tensor("out", [B_LOC, PLEN, D], BF16, kind="ExternalOutput").ap()

    from contextlib import ExitStack

    with tile.TileContext(nc) as tc, ExitStack() as ctx:
        _emit(ctx, tc, P_in, P8_in, wb_in, wc_in, w_mlp, w8_mlp, b_mlp, out)
    nc.compile()
    _NC_CACHE["nc"] = nc
    return nc


def run(inputs, trace=False, tmpdir=None):
    nc = _build()
    bf = ml_dtypes.bfloat16
    e4 = ml_dtypes.float8_e4m3
    Pf = np.ascontiguousarray(np.asarray(inputs["P"], dtype=np.float32))
    P = Pf.astype(bf)
    P8 = Pf.astype(e4)
    w_att = np.asarray(inputs["w_itr_att"], np.float32)
    shared = {
        "wb": np.ascontiguousarray(w_att[D:2 * D]),
        "wc": np.ascontiguousarray(w_att[2 * D:3 * D]),
        "w1": np.ascontiguousarray(np.asarray(inputs["w1"], np.float32)).astype(bf),
        "w2": np.ascontiguousarray(np.asarray(inputs["w2"], np.float32)).astype(bf),
        "w3": np.ascontiguousarray(np.asarray(inputs["w3"], np.float32)).astype(bf),
        "w81": np.ascontiguousarray(np.asarray(inputs["w1"], np.float32)[D:]).astype(e4),
        "w82": np.ascontiguousarray(np.asarray(inputs["w2"], np.float32)[D:]).astype(e4),
        "w83": np.ascontiguousarray(np.asarray(inputs["w3"], np.float32)[D:]).astype(e4),
        "b1": np.ascontiguousarray(np.asarray(inputs["b1"], np.float32)),
        "b2": np.ascontiguousarray(np.asarray(inputs["b2"], np.float32)),
        "b3": np.ascontiguousarray(np.asarray(inputs["b3"], np.float32)),
    }
    in_maps = [
        {"p_in": P[c * B_LOC : (c + 1) * B_LOC],
         "p8_in": P8[c * B_LOC : (c + 1) * B_LOC], **shared}
        for c in range(N_CORES)
    ]
    res = run_bass_kernel_spmd(nc, in_maps, list(range(N_CORES)), trace=trace,
                               tmpdir=tmpdir)
    full = np.concatenate(
        [np.asarray(res.results[c]["out"]).astype(np.float32)
         for c in range(N_CORES)], axis=0)
    return full, res


def kernel(**inputs):
    full, _ = run(inputs)
    return full
